# revision 1
# baseline (speedup 1.0000x reference)
"""Trainium2 Bass kernel: causal self-attention (modded-nanogpt style),
tensor-parallel over heads across 8 NeuronCores, AllToAll re-shard (split in
two query-halves so the output projection overlaps the second collective).

Self-contained: hardcodes B=1, T=4096, D=1024, H=8, Hd=128, scale=0.12.

Per-core program, processed per group of four 128-row tiles (2 pairs):
  qkv_pair(g,h)  2x[ 8 qkv matmuls into PSUM ]; v lambda-mix (Pool STT from
                 PSUM); sum-of-squares via DVE tensor_tensor_reduce from
                 PSUM; rsqrt via integer magic + 2 Newton steps on [P,4];
                 fused normalize+evict (PSUM -> f16 SBUF tensor_scalar);
                 pair-batched RoPE; PE transposes -> kT/qT
  attn_part(c,h) h=0: the 4 diagonal key-blocks (causal mask via Pool
                 affine_select, first block doubles as the masked copy into
                 acc); h=1: off-diagonal blocks. exp on ACT out of PSUM with
                 2^-12 bias; denominator adds on DVE; y^T += v.T @ expT
  flush pipeline: ones-matmul row-sum -> ACT evict -> DMA gather [1,512] ->
                 [128,4] -> batched DVE reciprocal -> DMA scatter back ->
                 Pool partition_broadcast -> DVE normalize -> ship halves
Then two AllToAlls (query-halves) re-shard heads -> sequence; the output
projection for the first half overlaps the second collective.
"""

import os
import sys

sys.path.insert(0, "/opt/trn_rl_repo")

from contextlib import ExitStack

import numpy as np

import concourse.bass as bass
import concourse.bacc as bacc
import concourse.mybir as mybir
import concourse.tile as tile
from concourse.bass_utils import run_bass_kernel_spmd
from concourse.masks import make_identity

N_CORES = 8
T = 4096
D = 1024
H = 8
HD = 128
ATTN_SCALE = 0.12
P = 128
TCH = 512
NT = T // P          # 32 t-tiles
NC_CH = T // TCH     # 8 chunks / tile groups
SHARD = T // N_CORES
QUARTER = HD // 4
HTCH = TCH // 2      # 256: query half shipped per collective

F32 = mybir.dt.float32
I32 = mybir.dt.int32
_MODE = os.environ.get("KBASS_MM_DT", "f16")
MMD = {"f32r": mybir.dt.float32r, "f16": mybir.dt.float16,
       "f32": F32}[_MODE]
NP_MMD = {"f32r": np.float32, "f16": np.float16, "f32": np.float32}[_MODE]
# exp(s - 12*ln2) = 2^-12 * exp(s): keeps fp16 exp values and their fp16
# partial sums in range; the scaling cancels in the softmax normalize.
EXP_BIAS = -8.317766166719343 if _MODE == "f16" else 0.0
RSQRT_MAGIC = 0x5F3759DF
RECIP_MODE = os.environ.get("KBASS_RECIP", "ln")    # ln | dve
SPLIT_CC = os.environ.get("KBASS_SPLIT_CC", "0") == "1"
BATCH_ROPE = os.environ.get("KBASS_ROPE", "batch") == "batch"
AFFINE_MASK = os.environ.get("KBASS_AFFINE", "1") == "1"
SSQ_MODE = os.environ.get("KBASS_SSQ", "stt")       # stt | ttr | act
# ttr (tensor_tensor_reduce) faults real TRN2 hw despite passing CoreSim

_cached = {}


def build_module():
    nc = bacc.Bacc("TRN2", target_bir_lowering=False, debug=False,
                   num_devices=N_CORES)

    x_t = nc.dram_tensor("x_t", [D, T], MMD, kind="ExternalInput")
    w_qkv = nc.dram_tensor("w_qkv", [D, 3 * HD], MMD, kind="ExternalInput")
    cos_t = nc.dram_tensor("cos_t", [T, QUARTER], MMD, kind="ExternalInput")
    sin_t = nc.dram_tensor("sin_t", [T, QUARTER], MMD, kind="ExternalInput")
    ve_h = nc.dram_tensor("ve_h", [T, HD], F32, kind="ExternalInput")
    lam = nc.dram_tensor("lam", [P, 2], F32, kind="ExternalInput")
    cpw = nc.dram_tensor("cpw", [D, D], MMD, kind="ExternalInput")
    y_shard = nc.dram_tensor("y_shard", [SHARD, D], F32, kind="ExternalOutput")

    with tile.TileContext(nc) as tc, nc.allow_low_precision(
            reason="reduced-precision matmul operands"), ExitStack() as ctx:
        const = ctx.enter_context(tc.tile_pool(name="const", bufs=1))
        wqkv_pool = ctx.enter_context(tc.tile_pool(name="wqkv", bufs=1))
        big = ctx.enter_context(tc.tile_pool(name="big", bufs=1))
        xt_pool = ctx.enter_context(tc.tile_pool(name="xt", bufs=7))
        cs_pool = ctx.enter_context(tc.tile_pool(name="cs", bufs=3))
        ve_pool = ctx.enter_context(tc.tile_pool(name="vein", bufs=3))
        scr_pool = ctx.enter_context(tc.tile_pool(name="scr", bufs=3))
        stat_pool = ctx.enter_context(tc.tile_pool(name="stat", bufs=3))
        qkn_pool = ctx.enter_context(tc.tile_pool(name="qkn", bufs=3))
        exp_pool = ctx.enter_context(tc.tile_pool(name="exp", bufs=6))
        acc_pool = ctx.enter_context(tc.tile_pool(name="acc", bufs=2))
        rro_pool = ctx.enter_context(tc.tile_pool(name="rro", bufs=2))
        cpw_pool = ctx.enter_context(tc.tile_pool(name="cpw", bufs=16))
        # PSUM: qkv 2 banks, s/transpose/cproj 3, y 2, r 1  (8 total)
        ps_qkv_pool = ctx.enter_context(
            tc.tile_pool(name="psqkv", bufs=2, space="PSUM"))
        ps_s = ctx.enter_context(tc.tile_pool(name="pss", bufs=3,
                                              space="PSUM"))
        ps_y_pool = ctx.enter_context(tc.tile_pool(name="psy", bufs=2,
                                                   space="PSUM"))
        ps_r_pool = ctx.enter_context(tc.tile_pool(name="psr", bufs=1,
                                                   space="PSUM"))
        dram = ctx.enter_context(tc.tile_pool(name="dram", bufs=1,
                                              space="DRAM"))

        # ---- critical DMAs first so their queues lead ----
        xt_tiles = {}

        def ensure_xt(i):  # i even: tile pair (i, i+1)
            if i in xt_tiles or i >= NT:
                return
            xt = xt_pool.tile([P, D // P, 2 * P], MMD, tag="xt",
                              name=f"xt{i}")
            nc.sync.dma_start(
                out=xt[:],
                in_=x_t.ap().rearrange("(k p) t -> p k t", p=P)
                    [:, :, i * P:(i + 2) * P])
            xt_tiles[i] = xt

        ensure_xt(0)
        ensure_xt(2)
        wqkv_sb = wqkv_pool.tile([P, D // P, 3 * HD], MMD)
        for k in range(D // P):
            eng = nc.scalar if k < 6 else nc.sync
            eng.dma_start(out=wqkv_sb[:, k, :],
                          in_=w_qkv.ap()[k * P:(k + 1) * P, :])
        lam_sb = const.tile([P, 2], F32)
        nc.scalar.dma_start(out=lam_sb[:], in_=lam.ap())

        # ---- constants ----
        ones_f = const.tile([P, 1], F32)
        nc.vector.memset(ones_f[:], 1.0)
        ones_col = const.tile([P, 1], MMD)
        nc.scalar.copy(ones_col[:], ones_f[:])
        expb_col = const.tile([P, 1], F32)
        nc.vector.memset(expb_col[:], EXP_BIAS)
        ident_f = const.tile([P, P], F32)
        make_identity(nc, ident_f)
        ident = const.tile([P, P], MMD)
        nc.scalar.copy(ident[:], ident_f[:])
        masks = []
        if not AFFINE_MASK:
            mk_f = const.tile([P, TCH], F32)
            for m in range(4):
                nc.vector.memset(mk_f[:], 1.0)
                nc.gpsimd.affine_select(
                    out=mk_f[:], in_=mk_f[:],
                    compare_op=mybir.AluOpType.is_ge, fill=0.0,
                    base=-P * m, channel_multiplier=-1, pattern=[[1, TCH]])
                mk = const.tile([P, TCH], MMD, name=f"mask{m}")
                nc.scalar.copy(mk[:], mk_f[:])
                masks.append(mk)

        # ---- persistent per-block tensors ----
        kT_t = [big.tile([P, P], MMD, name=f"kT{j}") for j in range(NT)]
        v_t = [big.tile([P, HD], MMD, name=f"v{j}") for j in range(NT)]
        qT_c = [big.tile([P, TCH], MMD, name=f"qT{c}") for c in range(NC_CH)]
        yT_c = [big.tile([P, TCH], MMD, name=f"yT{c}") for c in range(NC_CH)]

        if SPLIT_CC:
            cc_in_A = dram.tile([N_CORES * P * HTCH], MMD)
            cc_in_B = dram.tile([N_CORES * P * HTCH], MMD)
            cc_out_A = dram.tile([N_CORES * P * HTCH], MMD)
            cc_out_B = dram.tile([N_CORES * P * HTCH], MMD)
            cc_in_Av = cc_in_A[:].rearrange("(j p f) -> j p f",
                                            j=N_CORES, p=P)
            cc_in_Bv = cc_in_B[:].rearrange("(j p f) -> j p f",
                                            j=N_CORES, p=P)
            cc_out_Av = cc_out_A[:].rearrange("(j p f) -> j p f",
                                              j=N_CORES, p=P)
            cc_out_Bv = cc_out_B[:].rearrange("(j p f) -> j p f",
                                              j=N_CORES, p=P)
        else:
            cc_in = dram.tile([N_CORES * P * TCH], MMD)
            cc_out = dram.tile([N_CORES * P * TCH], MMD)
            cc_in_v = cc_in[:].rearrange("(j p f) -> j p f", j=N_CORES, p=P)
            cc_out_v = cc_out[:].rearrange("(j p f) -> j p f", j=N_CORES, p=P)

        # ---- flush: per chunk evict denominator row + un-normalized yT
        # out of PSUM (both plain Copy ops - no ACT table switch). The
        # reciprocal exp(-ln(d)) is deferred to one batch at the end so
        # the Ln<->Exp activation-table switch happens once, not per chunk.
        # KBASS_RECIP=dve instead does 4 quarter reciprocals per chunk.
        N_FLUSH = 2 if RECIP_MODE == "ln" else 6
        chunk_rows = {}   # c -> rrow (denominator row, SBUF f32)
        chunk_ysb = {}    # c -> un-normalized yT (SBUF f16)

        def ship(c, yT):
            if SPLIT_CC:
                nc.gpsimd.dma_start(out=cc_in_Av[c], in_=yT[:, 0:HTCH])
                nc.gpsimd.dma_start(out=cc_in_Bv[c], in_=yT[:, HTCH:TCH])
            else:
                nc.gpsimd.dma_start(out=cc_in_v[c], in_=yT[:])

        pending_den = []  # entries: [step, c, ps_y, ps_r, rrec, rb]

        def flush_step():
            if not pending_den:
                return
            ent = pending_den[0]
            step, c = ent[0], ent[1]
            if RECIP_MODE == "ln":
                if step == 0:
                    rrow = big.tile([1, TCH], F32, name=f"rrow{c}")
                    nc.scalar.copy(rrow[:], ent[3][0:1, :])
                    chunk_rows[c] = rrow
                else:
                    ysb = big.tile([P, TCH], MMD, name=f"ysb{c}")
                    nc.vector.tensor_copy(ysb[:], ent[2][:])
                    chunk_ysb[c] = ysb
                    pending_den.pop(0)
                    return
            else:
                if step == 0:
                    ent[4] = rro_pool.tile([1, TCH], F32, tag="rrec",
                                           name=f"rrec{c}")
                if step < 4:
                    q0 = step * (TCH // 4)
                    nc.vector.reciprocal(ent[4][0:1, q0:q0 + TCH // 4],
                                         ent[3][0:1, q0:q0 + TCH // 4])
                elif step == 4:
                    rb = rro_pool.tile([P, TCH], F32, tag="rb")
                    nc.gpsimd.partition_broadcast(rb[:], ent[4][:])
                    ent[5] = rb
                else:
                    nc.vector.tensor_mul(yT_c[c][:], ent[2][:], ent[5][:])
                    ship(c, yT_c[c])
                    pending_den.pop(0)
                    return
            ent[0] += 1

        def flush_all():
            while pending_den:
                flush_step()
            if RECIP_MODE != "ln":
                return
            # batched reciprocal: one Ln/Exp table visit for all chunks.
            # tile_wait_until pins this after every attention Exp so the
            # scheduler can't interleave it (each interleave costs two
            # 1.28us ACT table reloads).
            order = list(chunk_rows)
            recs = {}
            with tc.tile_wait_until(0.5):
                for c in order:
                    rln = rro_pool.tile([1, TCH], F32, tag="rln",
                                        name=f"rln{c}", bufs=NC_CH)
                    nc.scalar.activation(rln[:], chunk_rows[c][:],
                                         mybir.ActivationFunctionType.Ln)
                    recs[c] = rln
                for c in order:
                    rrec = rro_pool.tile([1, TCH], F32, tag="rrecd",
                                         name=f"rrecd{c}", bufs=NC_CH)
                    nc.scalar.activation(rrec[:], recs[c][:],
                                         mybir.ActivationFunctionType.Exp,
                                         scale=-1.0)
                    rb = rro_pool.tile([P, TCH], F32, tag="rb", bufs=2)
                    nc.gpsimd.partition_broadcast(rb[:], rrec[:])
                    nc.vector.tensor_mul(yT_c[c][:], chunk_ysb[c][:], rb[:])
                    ship(c, yT_c[c])

        # ---- qkv: one pair of 128-row tiles ----
        group_state = {}  # g -> (ve_g, cos_g, sin_g)
        sq128 = float(np.sqrt(HD))

        def qkv_pair(g, h):
            if h == 0:
                ensure_xt(4 * g)
                ensure_xt(4 * g + 2)
                ve_g = ve_pool.tile([P, 4, HD], F32, tag="ve", name=f"ve{g}")
                nc.gpsimd.dma_start(
                    out=ve_g[:],
                    in_=ve_h.ap().rearrange("(n p) e -> p n e", p=P)
                        [:, 4 * g:4 * g + 4, :])
                cos_g = cs_pool.tile([P, 4, QUARTER], MMD, tag="cos",
                                     name=f"cos{g}")
                sin_g = cs_pool.tile([P, 4, QUARTER], MMD, tag="sin",
                                     name=f"sin{g}")
                nc.gpsimd.dma_start(
                    out=cos_g[:],
                    in_=cos_t.ap().rearrange("(n p) e -> p n e", p=P)
                        [:, 4 * g:4 * g + 4, :])
                nc.gpsimd.dma_start(
                    out=sin_g[:],
                    in_=sin_t.ap().rearrange("(n p) e -> p n e", p=P)
                        [:, 4 * g:4 * g + 4, :])
                group_state[g] = (ve_g, cos_g, sin_g)
            ve_g, cos_g, sin_g = group_state[g]

            i0 = 4 * g + 2 * h
            ps_pair = []
            for ii in range(2):
                i = i0 + ii
                xt_huge = xt_tiles[i - i % 2]
                xoff = (i % 2) * P
                ps_qkv = ps_qkv_pool.tile([P, 3 * HD], F32, tag="psqkv",
                                          name=f"psqkv{i}")
                for k in range(D // P):
                    nc.tensor.matmul(ps_qkv[:], xt_huge[:, k, xoff:xoff + P],
                                     wqkv_sb[:, k, :],
                                     start=(k == 0), stop=(k == D // P - 1))
                ps_pair.append(ps_qkv)

            # v lambda-mix straight out of PSUM (gpsimd can't read PSUM)
            for ii in range(2):
                i = i0 + ii
                nc.vector.scalar_tensor_tensor(
                    out=v_t[i][:], in0=ps_pair[ii][:, 2 * HD:3 * HD],
                    scalar=lam_sb[:, 0:1], in1=ve_g[:, 2 * h + ii, :],
                    op0=mybir.AluOpType.mult, op1=mybir.AluOpType.add)

            # evict q,k to SBUF f32 (frees PSUM early), ssq via DVE TTR
            qk_sb = []
            for ii in range(2):
                qs = qkn_pool.tile([P, 2 * HD], F32, tag="qksb",
                                   name=f"qksb{i0 + ii}")
                if ii:
                    nc.vector.tensor_copy(qs[:], ps_pair[ii][:, 0:2 * HD])
                else:
                    nc.scalar.copy(qs[:], ps_pair[ii][:, 0:2 * HD])
                qk_sb.append(qs)
            ssq = stat_pool.tile([P, 4], F32, tag="ssq", name=f"ssq{i0}")
            for ii in range(2):
                for qk in range(2):
                    sqs = scr_pool.tile([P, HD], MMD, tag="sqscr")
                    if SSQ_MODE == "ttr":
                        nc.vector.tensor_tensor_reduce(
                            out=sqs[:],
                            in0=qk_sb[ii][:, qk * HD:(qk + 1) * HD],
                            in1=qk_sb[ii][:, qk * HD:(qk + 1) * HD],
                            scale=1.0, scalar=0.0,
                            op0=mybir.AluOpType.mult,
                            op1=mybir.AluOpType.add,
                            accum_out=ssq[:, 2 * ii + qk:2 * ii + qk + 1])
                    elif SSQ_MODE == "stt":
                        nc.vector.scalar_tensor_tensor(
                            out=sqs[:],
                            in0=qk_sb[ii][:, qk * HD:(qk + 1) * HD],
                            scalar=1.0,
                            in1=qk_sb[ii][:, qk * HD:(qk + 1) * HD],
                            op0=mybir.AluOpType.mult,
                            op1=mybir.AluOpType.mult,
                            accum_out=ssq[:, 2 * ii + qk:2 * ii + qk + 1])
                    else:
                        nc.scalar.activation(
                            sqs[:], qk_sb[ii][:, qk * HD:(qk + 1) * HD],
                            mybir.ActivationFunctionType.Square,
                            accum_out=ssq[:, 2 * ii + qk:2 * ii + qk + 1])

            # rsq = 1/sqrt(ssq): integer magic + 2 Newton steps
            h_i = stat_pool.tile([P, 4], I32, tag="h_i")
            nc.vector.tensor_scalar(
                out=h_i[:], in0=ssq[:].bitcast(I32), scalar1=1, scalar2=None,
                op0=mybir.AluOpType.logical_shift_right)
            y0 = stat_pool.tile([P, 4], F32, tag="y0")
            nc.vector.tensor_scalar(
                out=y0[:].bitcast(I32), in0=h_i[:], scalar1=-1,
                scalar2=RSQRT_MAGIC,
                op0=mybir.AluOpType.mult, op1=mybir.AluOpType.add)
            t1 = stat_pool.tile([P, 4], F32, tag="t1")
            rsq = stat_pool.tile([P, 4], F32, tag="rsq", name=f"rsq{i0}")
            cur = y0
            for it, nxt in ((0, t1), (1, rsq)):
                tt = stat_pool.tile([P, 4], F32, tag=f"tt{it}")
                nc.vector.tensor_mul(tt[:], cur[:], cur[:])
                nc.vector.tensor_mul(tt[:], tt[:], ssq[:])
                nc.vector.tensor_scalar(
                    out=tt[:], in0=tt[:], scalar1=-0.5, scalar2=1.5,
                    op0=mybir.AluOpType.mult, op1=mybir.AluOpType.add)
                nc.vector.tensor_mul(nxt[:], cur[:], tt[:])
                cur = nxt

            # normalize q,k -> f16 qkn pair tile
            qkn = qkn_pool.tile([P, 2, 2 * HD], MMD, tag="qkn",
                                name=f"qkn{i0}")
            for ii in range(2):
                nc.vector.tensor_scalar(
                    out=qkn[:, ii, 0:HD], in0=qk_sb[ii][:, 0:HD],
                    scalar1=rsq[:, 2 * ii:2 * ii + 1],
                    scalar2=ATTN_SCALE * sq128,
                    op0=mybir.AluOpType.mult, op1=mybir.AluOpType.mult)
                nc.vector.tensor_scalar(
                    out=qkn[:, ii, HD:2 * HD], in0=qk_sb[ii][:, HD:2 * HD],
                    scalar1=rsq[:, 2 * ii + 1:2 * ii + 2], scalar2=sq128,
                    op0=mybir.AluOpType.mult, op1=mybir.AluOpType.mult)

            # rope: y1 = x1 c + x2 s ; y2 = x2 c - x1 s on the rotated
            # quarters of q and k together
            def rope(x1, x2, cb_, sb_, shape):
                a = scr_pool.tile(shape, MMD, tag="ropeA")
                b = scr_pool.tile(shape, MMD, tag="ropeB")
                c2 = scr_pool.tile(shape, MMD, tag="ropeC")
                d2 = scr_pool.tile(shape, MMD, tag="ropeD")
                nc.vector.tensor_mul(a[:], x1, cb_)
                nc.vector.tensor_mul(b[:], x2, sb_)
                nc.vector.tensor_mul(c2[:], x2, cb_)
                nc.vector.tensor_mul(d2[:], x1, sb_)
                nc.vector.tensor_add(x1, a[:], b[:])
                nc.vector.tensor_sub(x2, c2[:], d2[:])

            src = qkn[:]
            part_ap = list(src.ap[0])
            if BATCH_ROPE:
                # one shot for the pair: free dims [tile 2][qk 2][quarter]
                def rot_rng(col0):
                    return bass.AP(src.tensor, src.offset + col0,
                                   [part_ap, [2 * HD, 2], [HD, 2],
                                    [1, QUARTER]])

                cs_src = cos_g[:, 2 * h:2 * h + 2, :]
                sn_src = sin_g[:, 2 * h:2 * h + 2, :]

                def cs_b(ap3):
                    return bass.AP(ap3.tensor, ap3.offset,
                                   [list(ap3.ap[0]), list(ap3.ap[1]),
                                    [0, 2], list(ap3.ap[-1])])

                rope(rot_rng(0), rot_rng(2 * QUARTER),
                     cs_b(cs_src), cs_b(sn_src), [P, 2, 2, QUARTER])
            else:
                for ii in range(2):
                    def rot3(col0):
                        return bass.AP(src.tensor,
                                       src.offset + ii * 2 * HD + col0,
                                       [part_ap, [HD, 2], [1, QUARTER]])

                    cs2 = cos_g[:, 2 * h + ii, :]
                    sn2 = sin_g[:, 2 * h + ii, :]

                    def cs_b2(ap2):
                        return bass.AP(ap2.tensor, ap2.offset,
                                       [list(ap2.ap[0]), [0, 2],
                                        list(ap2.ap[-1])])

                    rope(rot3(0), rot3(2 * QUARTER),
                         cs_b2(cs2), cs_b2(sn2), [P, 2, QUARTER])

            if h == 1:
                ensure_xt(4 * g + 4)
                ensure_xt(4 * g + 6)
            return qkn

        def transposes_pair(g, h, qkn):
            # PE transposes into [e, t] layout; emitted after the attention
            # part so they don't head-of-line-block ready s/y matmuls while
            # the DVE norm chain finishes.
            i0 = 4 * g + 2 * h
            for ii in range(2):
                i = i0 + ii
                sub = (2 * h + ii) * P
                for ei, (src_ap, dst, c0) in enumerate(
                        ((qkn[:, ii, 0:HD], qT_c[g], sub),
                         (qkn[:, ii, HD:2 * HD], kT_t[i], 0))):
                    ps_tr = ps_s.tile([P, P], MMD, tag="ps")
                    nc.tensor.transpose(ps_tr[:], src_ap, ident[:])
                    if ei:
                        nc.vector.tensor_copy(dst[:, c0:c0 + P], ps_tr[:])
                    else:
                        nc.scalar.copy(dst[:, c0:c0 + P], ps_tr[:])

        # ---- attention: chunk c processed in two parts ----
        attn_state = {}

        def attn_part(c, part):
            if part == 0:
                order = list(range(4 * c, 4 * c + 4)) + list(range(0, 4 * c))
                st = {"order": order, "pos": 0, "s": {},
                      "ps_y": ps_y_pool.tile([P, TCH], F32, tag="psy",
                                             name=f"psy{c}"),
                      "acc": acc_pool.tile([P, TCH], MMD, name=f"acc{c}")}
                attn_state[c] = st
            st = attn_state[c]
            order, s_psums = st["order"], st["s"]
            ps_y, acc = st["ps_y"], st["acc"]
            n = len(order)
            hi = 4 if part == 0 else n

            def s_mm(j):
                p_s = ps_s.tile([P, TCH], F32, tag="ps")
                nc.tensor.matmul(p_s[:], kT_t[j][:], qT_c[c][:],
                                 start=True, stop=True)
                return p_s

            while st["pos"] < hi:
                pos = st["pos"]
                j = order[pos]
                if pos == 0:
                    s_psums[j] = s_mm(j)
                if pos + 1 < n:
                    jn = order[pos + 1]
                    s_psums[jn] = s_mm(jn)
                flush_step()
                p_s = s_psums.pop(j)
                e_sb = exp_pool.tile([P, TCH], MMD)
                nc.scalar.activation(e_sb[:], p_s[:],
                                     mybir.ActivationFunctionType.Exp,
                                     bias=expb_col[:])
                m = j - 4 * c
                if 0 <= m < 4:
                    # diagonal: causal mask; first block doubles as the
                    # masked copy into acc
                    dst = acc if pos == 0 else e_sb
                    if AFFINE_MASK:
                        nc.gpsimd.affine_select(
                            out=dst[:], in_=e_sb[:],
                            compare_op=mybir.AluOpType.is_ge, fill=0.0,
                            base=-P * m, channel_multiplier=-1,
                            pattern=[[1, TCH]])
                    else:
                        nc.vector.tensor_mul(dst[:], e_sb[:], masks[m][:])
                    y_rhs = dst
                    if pos > 0:
                        nc.vector.tensor_add(acc[:], acc[:], e_sb[:])
                else:
                    nc.vector.tensor_add(acc[:], acc[:], e_sb[:])
                    y_rhs = e_sb
                nc.tensor.matmul(ps_y[:], v_t[j][:], y_rhs[:],
                                 start=(pos == 0), stop=(pos == n - 1))
                st["pos"] += 1

            if st["pos"] == n and "done" not in st:
                st["done"] = True
                ps_r = ps_r_pool.tile([1, TCH], F32, tag="psr",
                                      name=f"psr{c}")
                nc.tensor.matmul(ps_r[:], ones_col[:], acc[:],
                                 start=True, stop=True)
                pending_den.append([0, c, ps_y, ps_r, None, None])

        # ---- main loop ----
        cpw_tiles = {}
        for g in range(NC_CH):
            qkn0 = qkv_pair(g, 0)
            if g >= 2:
                attn_part(g - 1, 0)
            transposes_pair(g, 0, qkn0)
            qkn1 = qkv_pair(g, 1)
            if g >= 2:
                attn_part(g - 1, 1)
            transposes_pair(g, 1, qkn1)
            if g == 5:  # prefetch output-projection weights mid-flight
                for dh in range(D // TCH):
                    for hh in range(H):
                        ct = cpw_pool.tile([P, TCH], MMD, tag="cpw",
                                           name=f"cpw{hh}_{dh}")
                        nc.gpsimd.dma_start(
                            out=ct[:],
                            in_=cpw.ap()[hh * P:(hh + 1) * P,
                                         dh * TCH:(dh + 1) * TCH])
                        cpw_tiles[(hh, dh)] = ct
        attn_part(NC_CH - 1, 0)
        attn_part(NC_CH - 1, 1)
        attn_part(0, 0)
        attn_part(0, 1)
        flush_all()

        # ---- AllToAll (two query-halves when SPLIT_CC) ----
        yall = [big.tile([P, TCH], MMD, name=f"yall{j}")
                for j in range(N_CORES)]

        def cproj_phase(half):
            for i in (2 * half, 2 * half + 1):
                for dh in range(D // TCH):
                    ps_o = ps_s.tile([P, TCH], F32, tag="ps")
                    for hh in range(H):
                        nc.tensor.matmul(ps_o[:],
                                         yall[hh][:, i * P:(i + 1) * P],
                                         cpw_tiles[(hh, dh)][:],
                                         start=(hh == 0), stop=(hh == H - 1))
                    o_sb = exp_pool.tile([P, TCH], F32, tag="osb")
                    nc.scalar.copy(o_sb[:], ps_o[:])
                    nc.sync.dma_start(
                        out=y_shard.ap()[i * P:(i + 1) * P,
                                         dh * TCH:(dh + 1) * TCH],
                        in_=o_sb[:])

        if SPLIT_CC:
            nc.gpsimd.collective_compute(
                "AllToAll", mybir.AluOpType.bypass,
                replica_groups=[list(range(N_CORES))],
                ins=[cc_in_A[:].opt()], outs=[cc_out_A[:].opt()])
            nc.gpsimd.collective_compute(
                "AllToAll", mybir.AluOpType.bypass,
                replica_groups=[list(range(N_CORES))],
                ins=[cc_in_B[:].opt()], outs=[cc_out_B[:].opt()])
            for half, cc_v in ((0, cc_out_Av), (1, cc_out_Bv)):
                for j in range(N_CORES):
                    nc.sync.dma_start(
                        out=yall[j][:, half * HTCH:(half + 1) * HTCH],
                        in_=cc_v[j])
                cproj_phase(half)
        else:
            nc.gpsimd.collective_compute(
                "AllToAll", mybir.AluOpType.bypass,
                replica_groups=[list(range(N_CORES))],
                ins=[cc_in[:].opt()], outs=[cc_out[:].opt()])
            for j in range(N_CORES):
                nc.sync.dma_start(out=yall[j][:], in_=cc_out_v[j])
            cproj_phase(0)
            cproj_phase(1)

    nc.compile()
    return nc


def _host_prep(x, ve, qkv_w, lambdas, c_proj_w):
    x = np.asarray(x, dtype=np.float32)
    ve = np.asarray(ve, dtype=np.float32)
    qkv_w = np.asarray(qkv_w, dtype=np.float32)
    lambdas = np.asarray(lambdas, dtype=np.float32)
    c_proj_w = np.asarray(c_proj_w, dtype=np.float32)

    xT = np.ascontiguousarray(x[0].T.astype(NP_MMD))
    cpwT = np.ascontiguousarray(c_proj_w.T.astype(NP_MMD))
    lam_b = np.ascontiguousarray(np.broadcast_to(lambdas, (P, 2)))

    angular = (np.float32(1.0 / 1024.0)
               ** np.linspace(0.0, 1.0, QUARTER, dtype=np.float32))
    t = np.arange(T, dtype=np.float32)
    theta = t[:, None] * angular[None, :]
    cos32 = np.cos(theta).astype(NP_MMD)
    sin32 = np.sin(theta).astype(NP_MMD)

    in_maps = []
    for h in range(N_CORES):
        sl = slice(h * HD, (h + 1) * HD)
        w_qkvT = np.ascontiguousarray(np.concatenate(
            [qkv_w[0, sl, :].T, qkv_w[1, sl, :].T, qkv_w[2, sl, :].T],
            axis=1).astype(NP_MMD))
        in_maps.append({
            "x_t": xT,
            "w_qkv": w_qkvT,
            "cos_t": cos32,
            "sin_t": sin32,
            "ve_h": np.ascontiguousarray(ve[0][:, sl] * lambdas[1]),
            "lam": lam_b,
            "cpw": cpwT,
        })
    return in_maps


def kernel(x, ve, qkv_w, lambdas, c_proj_w, _trace=False, _trace_kwargs=None):
    if "nc" not in _cached:
        _cached["nc"] = build_module()
    nc = _cached["nc"]
    in_maps = _host_prep(x, ve, qkv_w, lambdas, c_proj_w)
    kw = {}
    if _trace:
        kw = dict(trace=True, **(_trace_kwargs or {}))
    res = run_bass_kernel_spmd(nc, in_maps, core_ids=list(range(N_CORES)),
                               **kw)
    _cached["last_result"] = res
    out = np.concatenate([res.results[c]["y_shard"] for c in range(N_CORES)],
                         axis=0)
    return out[None].astype(np.float32)

